# revision 1
# baseline (speedup 1.0000x reference)
"""Trainium2 Bass kernel: per-edge dot product (u_dot_v GNN predictor).

score[e] = sum_d h[src[e], d] * h[dst[e], d]   -> [E, 1] float32

Strategy (edge-parallel over 8 cores):
  - Each core gets E/8 = 80000 edges; the full node table h stays in HBM and
    rows are fetched per edge with the GPSIMD dma_gather instruction.
    The kernel is bound by the Q7 descriptor-generation rate (~8 ns/descriptor),
    so the design minimizes descriptor count.
  - dma_gather indices are int16, so node ids >= 32768 can't be addressed
    directly. Host buckets each core's edges 4 ways by (src >= 32768,
    dst >= 32768); each bucket's gathers use base-offset views of h with
    rebased indices.
  - Descriptor reduction: within each bucket, edges whose src rows are
    CONSECUTIVE (r, r+1) are paired; one elem_size=256/elem_step=128
    descriptor fetches both rows (overlapping-window source AP), so a pair
    of edges costs 1 src descriptor instead of 2. ~70% of edges pair up.
  - Per tile of 1024 descriptors: gather h[src] (paired or single) and the
    matching h[dst] rows, DVE multiply + segmented reduce over the feature
    axis -> [128, 8] scores per 1024 edge-slots.
  - Host un-permutes the bucketed scores back to edge order.
"""

import numpy as np

import concourse.bacc as bacc
import concourse.mybir as mybir
import concourse.tile as tile
from concourse import bass
from concourse.bass_utils import run_bass_kernel_spmd

N_NODES = 50000
D = 128
N_EDGES = 640000
N_CORES = 8
P = 128
E_CORE = N_EDGES // N_CORES  # 80000
NI = 1024                    # descriptors per dma_gather tile
C = NI // P                  # 8 chunks per partition
S = NI // 16                 # idx columns per 16-partition block
OFF = 32768                  # int16 index range boundary

_CACHE: dict = {}


def _h_tables(nc, h):
    """Per-bucket source APs: (single-row table, overlapping pair table)."""
    lo_rows, hi_rows = OFF, N_NODES - OFF
    h_lo1 = h[:OFF, :]
    h_hi1 = h[OFF:, :]
    # overlapping-window pair tables: pseudo-row r = elements 128r..128r+255
    # (rows r and r+1). Base stride 128 elems, window 256 elems.
    h_lo2 = bass.AP(h.tensor, 0, [[D, lo_rows - 1], [1, 2 * D]])
    h_hi2 = bass.AP(h.tensor, OFF * D, [[D, hi_rows - 1], [1, 2 * D]])
    return (h_lo1, h_hi1, h_lo2, h_hi2)


def _build(cfg):
    """cfg = (t2 per group [4], t1 per group [4]) tile counts."""
    t2g, t1g = cfg
    T2, T1 = sum(t2g), sum(t1g)
    nc = bacc.Bacc(
        "TRN2",
        target_bir_lowering=False,
        debug=False,
        enable_asserts=False,
        num_devices=N_CORES,
    )
    h = nc.dram_tensor("h", [N_NODES, D], mybir.dt.float32, kind="ExternalInput").ap()

    def idx_in(name, ntiles):
        return nc.dram_tensor(name, [P, max(ntiles, 1) * S], mybir.dt.int16,
                              kind="ExternalInput").ap()

    sp = idx_in("sp", T2)    # pair src bases
    da = idx_in("da", T2)    # pair dst idx, first edge of pair
    db = idx_in("db", T2)    # pair dst idx, second edge
    ss = idx_in("ss", T1)    # single src idx
    ds = idx_in("ds", T1)    # single dst idx
    ncols = (2 * T2 + T1) * C
    out = nc.dram_tensor("out", [P, ncols], mybir.dt.float32, kind="ExternalOutput").ap()

    h_lo1, h_hi1, h_lo2, h_hi2 = _h_tables(nc, h)
    tab1 = [h_lo1, h_lo1, h_hi1, h_hi1]   # src single table per group
    tab2 = [h_lo2, h_lo2, h_hi2, h_hi2]   # src pair table per group
    dtab = [h_lo1, h_hi1, h_lo1, h_hi1]   # dst table per group

    with tile.TileContext(nc) as tc:
        with (
            tc.tile_pool(name="idx", bufs=1) as ipool,
            tc.tile_pool(name="gath", bufs=4) as gpool,
            tc.tile_pool(name="res", bufs=1) as rpool,
        ):
            def load_idx(ap_dram, ntiles, tag):
                t = ipool.tile([P, max(ntiles, 1) * S], mybir.dt.int16, tag=tag)
                nc.sync.dma_start(out=t[:], in_=ap_dram)
                return t

            sp_sb = load_idx(sp, T2, "sp")
            da_sb = load_idx(da, T2, "da")
            db_sb = load_idx(db, T2, "db")
            ss_sb = load_idx(ss, T1, "ss")
            ds_sb = load_idx(ds, T1, "ds")
            out_sb = rpool.tile([P, ncols], mybir.dt.float32)

            # pair tiles
            t2 = 0
            for g in range(4):
                for _ in range(t2g[g]):
                    isl = slice(t2 * S, (t2 + 1) * S)
                    hu = gpool.tile([P, 2 * NI], mybir.dt.float32, tag="hu2")
                    hva = gpool.tile([P, NI], mybir.dt.float32, tag="hva")
                    hvb = gpool.tile([P, NI], mybir.dt.float32, tag="hvb")
                    nc.gpsimd.dma_gather(
                        out_ap=hu[:].rearrange("p (c d) -> p c d", d=2 * D),
                        in_ap=tab2[g], idxs_ap=sp_sb[:, isl],
                        num_idxs=NI, num_idxs_reg=NI,
                        elem_size=2 * D, elem_step=D,
                    )
                    nc.gpsimd.dma_gather(
                        out_ap=hva[:].rearrange("p (c d) -> p c d", d=D),
                        in_ap=dtab[g], idxs_ap=da_sb[:, isl],
                        num_idxs=NI, num_idxs_reg=NI, elem_size=D,
                    )
                    nc.gpsimd.dma_gather(
                        out_ap=hvb[:].rearrange("p (c d) -> p c d", d=D),
                        in_ap=dtab[g], idxs_ap=db_sb[:, isl],
                        num_idxs=NI, num_idxs_reg=NI, elem_size=D,
                    )
                    hu3 = hu[:].rearrange("p (c d) -> p c d", d=2 * D)
                    nc.vector.tensor_mul(out=hva[:], in0=hva[:],
                                         in1=hu3[:, :, :D])
                    nc.vector.tensor_mul(out=hvb[:], in0=hvb[:],
                                         in1=hu3[:, :, D:])
                    nc.vector.tensor_reduce(
                        out=out_sb[:, (2 * t2) * C:(2 * t2 + 1) * C],
                        in_=hva[:].rearrange("p (c d) -> p c d", d=D),
                        axis=mybir.AxisListType.X, op=mybir.AluOpType.add)
                    nc.vector.tensor_reduce(
                        out=out_sb[:, (2 * t2 + 1) * C:(2 * t2 + 2) * C],
                        in_=hvb[:].rearrange("p (c d) -> p c d", d=D),
                        axis=mybir.AxisListType.X, op=mybir.AluOpType.add)
                    t2 += 1

            # single tiles
            t1 = 0
            for g in range(4):
                for _ in range(t1g[g]):
                    isl = slice(t1 * S, (t1 + 1) * S)
                    hu = gpool.tile([P, NI], mybir.dt.float32, tag="hu1")
                    hv = gpool.tile([P, NI], mybir.dt.float32, tag="hv1")
                    nc.gpsimd.dma_gather(
                        out_ap=hu[:].rearrange("p (c d) -> p c d", d=D),
                        in_ap=tab1[g], idxs_ap=ss_sb[:, isl],
                        num_idxs=NI, num_idxs_reg=NI, elem_size=D,
                    )
                    nc.gpsimd.dma_gather(
                        out_ap=hv[:].rearrange("p (c d) -> p c d", d=D),
                        in_ap=dtab[g], idxs_ap=ds_sb[:, isl],
                        num_idxs=NI, num_idxs_reg=NI, elem_size=D,
                    )
                    nc.vector.tensor_mul(out=hu[:], in0=hu[:], in1=hv[:])
                    nc.vector.tensor_reduce(
                        out=out_sb[:, (2 * T2 + t1) * C:(2 * T2 + t1 + 1) * C],
                        in_=hu[:].rearrange("p (c d) -> p c d", d=D),
                        axis=mybir.AxisListType.X, op=mybir.AluOpType.add)
                    t1 += 1
            nc.sync.dma_start(out=out, in_=out_sb[:])
    nc.compile()
    return nc


def _get_nc(cfg):
    key = (tuple(cfg[0]), tuple(cfg[1]))
    nc = _CACHE.get(key)
    if nc is None:
        nc = _build(key)
        _CACHE[key] = nc
    return nc


def _pair_decompose(s, d, eids):
    """Greedy consecutive-row pairing of one bucket's edges.

    Returns (pa, pb, singles): edge-id arrays; s[pb] == s[pa] + 1."""
    o = np.argsort(s, kind="stable")
    ss = s[o]
    rows, starts, cnts = np.unique(ss, return_index=True, return_counts=True)
    pa, pb, singles = [], [], []
    carry = np.empty(0, dtype=np.int64)
    prev = -2
    for r, st, c in zip(rows.tolist(), starts.tolist(), cnts.tolist()):
        cur = o[st:st + c]
        if r == prev + 1 and len(carry):
            m = min(len(carry), c)
            pa.append(carry[:m])
            pb.append(cur[:m])
            if len(carry) > m:
                singles.append(carry[m:])
            carry = cur[m:]
        else:
            if len(carry):
                singles.append(carry)
            carry = cur
        prev = r
    if len(carry):
        singles.append(carry)
    cat = lambda lst: (np.concatenate(lst) if lst else np.empty(0, dtype=np.int64))
    pa, pb, singles = cat(pa), cat(pb), cat(singles)
    return eids[pa], eids[pb], eids[singles]


def _wrap_idx(vals, ntiles):
    """[ntiles*NI] int array -> [128, ntiles*S] int16 wrapped layout."""
    v16 = vals.astype(np.uint16).view(np.int16).reshape(ntiles, S, 16)
    blk = v16.transpose(2, 0, 1).reshape(16, ntiles * S)
    return np.tile(blk, (8, 1))


def _prepare_core(s, d):
    """Bucket + pair-decompose one core's edges.

    Returns dict with idx arrays (unpadded, per group) and bookkeeping."""
    grp = (s >= OFF).astype(np.int8) * 2 + (d >= OFF).astype(np.int8)
    per_group = []
    for g in range(4):
        eids = np.where(grp == g)[0]
        sg = s[eids] - OFF * (g >> 1)
        pa, pb, single = _pair_decompose(sg, None, eids)
        per_group.append({
            "pa": pa, "pb": pb, "single": single,
            "soff": OFF * (g >> 1), "doff": OFF * (g & 1),
        })
    return per_group


def _core_arrays(s, d, per_group, t2g, t1g):
    """Build padded idx arrays for one core given global tile counts."""
    T2, T1 = sum(t2g), sum(t1g)
    sp = np.zeros(max(T2, 1) * NI, dtype=np.int32)
    da = np.zeros(max(T2, 1) * NI, dtype=np.int32)
    db = np.zeros(max(T2, 1) * NI, dtype=np.int32)
    ss_ = np.zeros(max(T1, 1) * NI, dtype=np.int32)
    ds_ = np.zeros(max(T1, 1) * NI, dtype=np.int32)
    b2 = b1 = 0
    for g in range(4):
        pg = per_group[g]
        n2, n1 = len(pg["pa"]), len(pg["single"])
        sp[b2:b2 + n2] = s[pg["pa"]] - pg["soff"]
        da[b2:b2 + n2] = d[pg["pa"]] - pg["doff"]
        db[b2:b2 + n2] = d[pg["pb"]] - pg["doff"]
        ss_[b1:b1 + n1] = s[pg["single"]] - pg["soff"]
        ds_[b1:b1 + n1] = d[pg["single"]] - pg["doff"]
        b2 += t2g[g] * NI
        b1 += t1g[g] * NI
    return (
        _wrap_idx(sp, max(T2, 1)), _wrap_idx(da, max(T2, 1)),
        _wrap_idx(db, max(T2, 1)), _wrap_idx(ss_, max(T1, 1)),
        _wrap_idx(ds_, max(T1, 1)),
    )


def _unpermute_core(out, per_group, t2g, t1g):
    T2, T1 = sum(t2g), sum(t1g)
    ncols = (2 * T2 + T1) * C
    # slot j of pair tile t2 -> scores at out[j%128, (2*t2 + {0,1})*C + (j%NI)//128]
    res = np.empty(E_CORE, dtype=np.float32)
    o3 = out.reshape(P, ncols // C, C)
    b2 = b1 = 0
    for g in range(4):
        pg = per_group[g]
        n2, n1 = len(pg["pa"]), len(pg["single"])
        j = b2 + np.arange(n2)
        t_arr = j // NI
        res[pg["pa"]] = o3[j % P, 2 * t_arr, (j % NI) // P]
        res[pg["pb"]] = o3[j % P, 2 * t_arr + 1, (j % NI) // P]
        j1 = b1 + np.arange(n1)
        res[pg["single"]] = o3[j1 % P, 2 * T2 + j1 // NI, (j1 % NI) // P]
        b2 += t2g[g] * NI
        b1 += t1g[g] * NI
    return res


def kernel(h, src_idx, dst_idx):
    h = np.ascontiguousarray(np.asarray(h, dtype=np.float32))
    src = np.asarray(src_idx).astype(np.int32).reshape(N_CORES, E_CORE)
    dst = np.asarray(dst_idx).astype(np.int32).reshape(N_CORES, E_CORE)

    pgs = [_prepare_core(src[c], dst[c]) for c in range(N_CORES)]
    t2g = [0, 0, 0, 0]
    t1g = [0, 0, 0, 0]
    for c in range(N_CORES):
        for g in range(4):
            t2g[g] = max(t2g[g], -(-len(pgs[c][g]["pa"]) // NI))
            t1g[g] = max(t1g[g], -(-len(pgs[c][g]["single"]) // NI))
    cfg = (tuple(t2g), tuple(t1g))

    nc = _get_nc(cfg)
    in_maps = []
    for c in range(N_CORES):
        sp, da, db, ss_, ds_ = _core_arrays(src[c], dst[c], pgs[c], t2g, t1g)
        in_maps.append({"h": h, "sp": sp, "da": da, "db": db, "ss": ss_, "ds": ds_})
    res = run_bass_kernel_spmd(nc, in_maps, core_ids=list(range(N_CORES)))
    outs = [
        _unpermute_core(np.asarray(res.results[c]["out"], dtype=np.float32),
                        pgs[c], t2g, t1g)
        for c in range(N_CORES)
    ]
    return np.concatenate(outs).reshape(N_EDGES, 1)

